# revision 11
# baseline (speedup 1.0000x reference)
"""GCN edge-aggregation kernel for 8 Trainium2 NeuronCores.

Math (see nn_GCNEdge): h = relu((segment_sum(edge_data, dst) / max(count,1)) @ W.T + b)

Strategy (v3 — bf16 payload, 32-node groups, PE column tiling, mixed chunk
profiles for ~1.6% padding)
-------------------------------------------------------------------------
Host-side (sharding/layout only):
  * 100352 node slots = 8 cores x 98 blocks x 4 groups x 32 nodes.  Nodes are
    assigned to groups by degree-balanced LPT packing against a *static*
    per-block chunk budget: blocks 0..87 get (4,4,4,4) chunks of 128 edges
    (all groups <=512 edges), blocks 88..97 get (4,5,4,5) (two loose groups
    <=640).  Total slots/core 203264 vs ~200.4K edges -> ~1.6% padding, and
    the chunk schedule is identical across cores -> one compiled program.
  * Edge features ship as single bf16 (full-chain rel err ~4.6e-3 vs the
    2e-2 gate).  Reciprocal mean weights (1/max(count,1)) are index-derived
    metadata and ship as a tiny [128 x 98] f32 tensor.
  * Per-core edge features are one [128 x 203264] bf16 HBM tensor whose
    partition rows are fully contiguous; input DMAs slice 7-block (~3.7 MB)
    windows mid-stream and single blocks at the head/tail so compute starts
    ~1.5us after launch and drains per-block at the end.

Device-side (per core, per 128-node block):
  * One-hot build on DVE is only 32 wide (is_equal against a tiled iota row):
    4x less DVE work than a 128-wide one-hot.
  * 16-18 matmuls accumulate oh_chunk.T @ x_chunk into a [128,128] f32 PSUM
    tile using PE 128x32 column tiling: each group owns PSUM partition
    quadrant 32g, so the 4 groups' weight loads overlap other groups'
    matmuls (round-robin emission).
  * ACT drains PSUM with the per-node reciprocal scale (mean), PE transposes
    4 blocks into a [128,512] tile, then out = relu(W @ agg.T + b) with a
    bf16 W; bf16 output columns staged [128,2048] and DMA'd per 16 blocks.

No collectives: output shards are disjoint; host inverts the node permutation.
"""

import numpy as np
import ml_dtypes

BF16 = ml_dtypes.bfloat16

N_NODES = 100000
N_EDGES = 1600000
F = 128
N_CORES = 8
BLOCKS = 98                        # blocks per core
N16 = 88                           # blocks with the (4,4,4,4) profile
GPB = 4                            # groups per block
GW = 32                            # nodes per group
K_MAX = 18
CHUNKS_16 = (4, 4, 4, 4)
CHUNKS_18 = (4, 5, 4, 5)
CBASE_16 = (0, 4, 8, 12)
CBASE_18 = (0, 4, 9, 13)
KB = [16] * N16 + [18] * (BLOCKS - N16)          # chunks per block
COLSTART = np.concatenate([[0], np.cumsum(np.array(KB) * 128)]).astype(np.int64)
CORE_COLS = int(COLSTART[-1])                    # 203264
LIDSTART = np.concatenate([[0], np.cumsum(KB)]).astype(np.int64)
LID_COLS = int(LIDSTART[-1])                     # 1588
NODES_PER_CORE = BLOCKS * 128                    # 12544
NPAD = N_CORES * NODES_PER_CORE                  # 100352
N_GROUPS = N_CORES * BLOCKS * GPB                # 3136
# input DMA plan: 4 head singles, 13 megas of 7 blocks, 3 tail singles
HEAD = [(b, b + 1) for b in range(4)]
MEGAS = [(4 + 7 * m, 11 + 7 * m) for m in range(13)]
TAIL = [(b, b + 1) for b in range(95, 98)]

_module_cache = {}


def _chunks(b):
    return (CHUNKS_16, CBASE_16) if b < N16 else (CHUNKS_18, CBASE_18)


def _build_module():
    import concourse.mybir as mybir
    import concourse.tile as tile
    from concourse import bacc

    f32 = mybir.dt.float32
    bf16 = mybir.dt.bfloat16

    nc = bacc.Bacc("TRN2", target_bir_lowering=False, debug=False)
    xe = nc.dram_tensor("xe", [128, CORE_COLS], bf16, kind="ExternalInput")
    lid = nc.dram_tensor("lid", [128, LID_COLS], bf16, kind="ExternalInput")
    recv = nc.dram_tensor("recv", [128, BLOCKS], f32, kind="ExternalInput")
    iotar = nc.dram_tensor("iotar", [128, K_MAX * GW], bf16, kind="ExternalInput")
    wt = nc.dram_tensor("wt", [128, 128], bf16, kind="ExternalInput")
    bias = nc.dram_tensor("bias", [128, 1], f32, kind="ExternalInput")
    ident = nc.dram_tensor("ident", [128, 128], f32, kind="ExternalInput")
    out = nc.dram_tensor("out", [128, BLOCKS * 128], bf16, kind="ExternalOutput")

    xe_ap = xe.ap()
    out_ap = out.ap()
    MEGA_W = 7 * 18 * 128            # widest mega tile

    with tile.TileContext(nc) as tc:
        with (
            tc.tile_pool(name="const", bufs=1) as cpool,
            tc.tile_pool(name="xp", bufs=3) as xpool,
            tc.tile_pool(name="ohp", bufs=4) as ohpool,
            tc.tile_pool(name="ep", bufs=4) as epool,
            tc.tile_pool(name="stp", bufs=2) as stpool,
            tc.tile_pool(name="psS", bufs=4, space="PSUM") as psS,
            tc.tile_pool(name="psT", bufs=2, space="PSUM") as psT,
            tc.tile_pool(name="psO", bufs=2, space="PSUM") as psO,
        ):
            # Startup order matters: the one-hot for block 0 needs only iotar
            # and the first 4 blocks' lid columns, and the first matmuls need
            # only block 0's edge slice — front-load those, stream the bulky
            # lid remainder behind the head singles.
            LID_HEAD = int(LIDSTART[len(HEAD)])
            iotar_t = cpool.tile([128, K_MAX * GW], bf16)
            nc.sync.dma_start(iotar_t[:], iotar.ap()[:])
            lidh_t = cpool.tile([128, LID_HEAD], bf16)
            nc.sync.dma_start(lidh_t[:], lid.ap()[:, 0:LID_HEAD])
            wt_t = cpool.tile([128, 128], bf16)
            bias_t = cpool.tile([128, 1], f32)
            id_t = cpool.tile([128, 128], f32)
            rec_t = cpool.tile([128, BLOCKS], f32)
            lidr_t = cpool.tile([128, LID_COLS - LID_HEAD], bf16)

            def emit_const_dmas():
                # Issued after the head edge singles: each dma_start costs
                # ~0.6us of Sync issue time, and none of these are needed
                # before iteration 2.
                nc.sync.dma_start(rec_t[:], recv.ap()[:])
                nc.sync.dma_start(wt_t[:], wt.ap()[:])
                nc.sync.dma_start(bias_t[:], bias.ap()[:])
                nc.sync.dma_start(id_t[:], ident.ap()[:])
                nc.sync.dma_start(lidr_t[:], lid.ap()[:, LID_HEAD:])

            where = {}               # block -> (tile, local col offset)

            def emit_span(name, tag, bufs, b0, b1, width):
                w = int(COLSTART[b1] - COLSTART[b0])
                xt = xpool.tile([128, width], bf16, name=name, tag=tag, bufs=bufs)
                nc.sync.dma_start(
                    xt[:, 0:w], xe_ap[:, int(COLSTART[b0]):int(COLSTART[b1])]
                )
                for b in range(b0, b1):
                    where[b] = (xt, int(COLSTART[b] - COLSTART[b0]))

            def emit_onehot(b):
                k = KB[b]
                if b < len(HEAD):
                    lsrc = lidh_t[:, int(LIDSTART[b]):int(LIDSTART[b]) + k]
                else:
                    o = int(LIDSTART[b]) - LID_HEAD
                    lsrc = lidr_t[:, o:o + k]
                oh = ohpool.tile([128, K_MAX * GW], bf16, name=f"oh{b}", tag="oh")
                nc.vector.tensor_tensor(
                    out=oh[:, 0:k * GW].rearrange("p (c f) -> p c f", c=k),
                    in0=iotar_t[:, 0:k * GW].rearrange("p (c f) -> p c f", c=k),
                    in1=lsrc.to_broadcast([128, k, GW]),
                    op=mybir.AluOpType.is_equal,
                )
                return oh

            def emit_mms(b, oh):
                ps = psS.tile([128, 128], f32, name=f"ps{b}", tag="ps")
                xt, xoff = where.pop(b)
                CH, CB = _chunks(b)
                for r in range(max(CH)):
                    for g in range(GPB):
                        if r >= CH[g]:
                            continue
                        c = CB[g] + r
                        nc.tensor.matmul(
                            ps[g * GW:(g + 1) * GW, :],
                            lhsT=oh[:, c * GW:(c + 1) * GW],
                            rhs=xt[:, xoff + c * 128: xoff + (c + 1) * 128],
                            start=(r == 0),
                            stop=(r == CH[g] - 1),
                            tile_position=(0, g * GW),
                        )
                return ps

            def emit_agg(b, ps):
                agg = epool.tile([128, 128], f32, name=f"agg{b}", tag="agg")
                nc.scalar.activation(
                    agg[:], ps[:],
                    mybir.ActivationFunctionType.Copy,
                    scale=rec_t[:, b:b + 1],
                )
                return agg

            stage = {}

            def emit_group(k, aggs):
                b0 = 4 * k
                nb = min(4, BLOCKS - b0)
                gw = nb * 128
                pT = psT.tile([128, 512], f32, name=f"pT{k}", tag="pT")
                for j in range(nb):
                    nc.tensor.transpose(
                        pT[:, j * 128:(j + 1) * 128], aggs[b0 + j][:], id_t[:]
                    )
                aggT = epool.tile([128, 512], bf16, name=f"aggT{k}", tag="aggT", bufs=2)
                nc.scalar.copy(aggT[:, 0:gw], pT[:, 0:gw])
                pO = psO.tile([128, 512], f32, name=f"pO{k}", tag="pO")
                nc.tensor.matmul(
                    pO[:, 0:gw], lhsT=wt_t[:], rhs=aggT[:, 0:gw],
                    start=True, stop=True,
                )
                s = k // 4
                if k % 4 == 0:
                    stage["t"] = stpool.tile([128, 2048], bf16, name=f"st{s}", tag="st")
                st = stage["t"]
                soff = (k % 4) * 512
                nc.scalar.activation(
                    st[:, soff:soff + gw], pO[:, 0:gw],
                    mybir.ActivationFunctionType.Relu,
                    bias=bias_t[:, 0:1], scale=1.0,
                )
                if k % 4 == 3 or b0 + nb == BLOCKS:
                    w = soff + gw
                    nc.sync.dma_start(out_ap[:, s * 2048: s * 2048 + w], st[:, 0:w])

            # Software-pipelined emission (strict in-order engine queues):
            #   iter b:  [input DMA per plan] | one-hot(b) | PE matmuls(b-1)
            #            | ACT mean-drain(b-2) | group stage per 4 blocks
            pend_oh = {}
            pend_ps = {}
            pend_agg = {}
            for b in range(BLOCKS):
                if b == 0:
                    for i, (h0, h1) in enumerate(HEAD):
                        emit_span(f"xh{i}", "xh", len(HEAD), h0, h1, 16 * 128)
                    emit_const_dmas()
                    emit_span("xm0", "xm", 3, *MEGAS[0], MEGA_W)
                elif (b + 3) % 7 == 0 and (b + 3) // 7 <= 12:
                    m = (b + 3) // 7
                    emit_span(f"xm{m}", "xm", 3, *MEGAS[m], MEGA_W)
                elif 88 <= b <= 90:
                    i = b - 88
                    emit_span(f"xz{i}", "xz", len(TAIL), *TAIL[i], 18 * 128)
                pend_oh[b] = emit_onehot(b)
                if b >= 1:
                    pend_ps[b - 1] = emit_mms(b - 1, pend_oh.pop(b - 1))
                if b >= 2:
                    pend_agg[b - 2] = emit_agg(b - 2, pend_ps.pop(b - 2))
                if b >= 5 and (b - 5) % 4 == 0:
                    k = (b - 5) // 4
                    emit_group(k, pend_agg)
                    for bb in range(4 * k, 4 * k + 4):
                        pend_agg.pop(bb)
            last = BLOCKS - 1
            pend_ps[last] = emit_mms(last, pend_oh.pop(last))
            for bb in sorted(pend_ps):
                pend_agg[bb] = emit_agg(bb, pend_ps.pop(bb))
            emit_group(24, pend_agg)

    nc.compile()
    return nc


def _get_module():
    if "m" not in _module_cache:
        _module_cache["m"] = _build_module()
    return _module_cache["m"]


def _group_meta():
    """Per-group (gid = ((core*98+blk)*4+g)) caps and in-block slot bases."""
    blk = np.arange(N_GROUPS) // GPB % BLOCKS
    g = np.arange(N_GROUPS) % GPB
    loose = (blk >= N16) & ((g == 1) | (g == 3))
    caps = np.where(loose, 640, 512).astype(np.int64)
    cb16 = np.asarray(CBASE_16)
    cb18 = np.asarray(CBASE_18)
    slot_base = np.where(blk < N16, cb16[g], cb18[g]) * 128
    return caps, slot_base, blk, g


def _pack_nodes(deg, caps):
    """Degree-balanced LPT assignment of NPAD node slots to 3136 groups."""
    import heapq

    order = np.argsort(-deg, kind="stable")
    bias = np.where(caps == 640, -60.0, 0.0)
    load = np.zeros(N_GROUPS)
    cnt = np.zeros(N_GROUPS, np.int32)
    heap = [(bias[j], j) for j in range(N_GROUPS)]
    heapq.heapify(heap)
    assign_g = np.empty(NPAD, np.int32)
    for n in order:
        d = deg[n]
        while True:
            _, j = heapq.heappop(heap)
            if cnt[j] < GW:
                break
        assign_g[n] = j
        cnt[j] += 1
        load[j] += d
        if cnt[j] < GW:
            heapq.heappush(heap, (load[j] + bias[j], j))

    # Repair pass (defensive; LPT stays under caps for this data):
    for _ in range(64):
        over = np.nonzero(load > caps)[0]
        if not len(over):
            break
        for j in over:
            members = np.nonzero(assign_g == j)[0]
            excess = load[j] - caps[j]
            victims = members[np.argsort(deg[members])]
            tgt = np.argsort(load + bias)
            for v in victims:
                if excess <= 0:
                    break
                for j2 in tgt[:16]:
                    if j2 == j:
                        continue
                    m2 = np.nonzero(assign_g == j2)[0]
                    small = m2[np.argmin(deg[m2])]
                    if deg[small] < deg[v] and load[j2] + deg[v] - deg[small] <= caps[j2]:
                        assign_g[v], assign_g[small] = j2, j
                        delta = deg[v] - deg[small]
                        load[j] -= delta
                        load[j2] += delta
                        excess -= delta
                        break
    assert (load <= caps).all(), "group packing infeasible"
    return assign_g


def prepare_inputs(edge_data, dst, W, b):
    """Host-side sharding: degree-balanced routing of edges to core/block/group."""
    edge_data = np.asarray(edge_data, dtype=np.float32)
    dst = np.asarray(dst).astype(np.int64)
    W = np.asarray(W, dtype=np.float32)
    b = np.asarray(b, dtype=np.float32)
    E = dst.shape[0]

    caps, slot_base, _, _ = _group_meta()
    deg = np.bincount(dst, minlength=NPAD).astype(np.int64)
    assign_g = _pack_nodes(deg, caps)

    node_order = np.argsort(assign_g, kind="stable")
    lcl = np.empty(NPAD, np.int32)
    lcl[node_order] = np.arange(NPAD, dtype=np.int32) % GW
    nodemap = node_order  # nodemap[gid*32 + l] = node id

    # edge -> slot
    gid_e = assign_g[dst]
    cnt_g = np.bincount(gid_e, minlength=N_GROUPS)
    starts = np.zeros(N_GROUPS, np.int64)
    np.cumsum(cnt_g[:-1], out=starts[1:])
    eorder = np.argsort(gid_e, kind="stable")
    rank = np.empty(E, np.int64)
    rank[eorder] = np.arange(E, dtype=np.int64) - np.repeat(starts, cnt_g)
    blk_glob = gid_e // GPB                       # core*98 + block
    core_e = blk_glob // BLOCKS
    bl_e = blk_glob % BLOCKS
    slot = core_e * CORE_COLS + COLSTART[bl_e] + slot_base[gid_e] + rank

    TOT = N_CORES * CORE_COLS
    X = np.zeros((TOT, 128), BF16)
    X[slot] = edge_data.astype(BF16)
    lid_f = np.full(TOT, -1.0, np.float32)
    lid_f[slot] = lcl[dst]

    # per-core [128, CORE_COLS] with partition rows contiguous:
    # region A: blocks < N16  [N16, 16, 128, 128] -> (2,0,1,3)
    # region B: blocks >= N16 [10, 18, 128, 128] -> (2,0,1,3)
    n_a = N16 * 16 * 128
    Xc = X.reshape(N_CORES, CORE_COLS, 128)
    lc = lid_f.reshape(N_CORES, CORE_COLS)
    xes, lids = [], []
    for c in range(N_CORES):
        A = Xc[c, :n_a].reshape(N16, 16, 128, 128).transpose(2, 0, 1, 3)
        Bq = Xc[c, n_a:].reshape(BLOCKS - N16, 18, 128, 128).transpose(2, 0, 1, 3)
        xes.append(np.concatenate(
            [A.reshape(128, N16 * 16 * 128), Bq.reshape(128, -1)], axis=1
        ))
        la = lc[c, :n_a].reshape(N16, 16, 128).transpose(2, 0, 1)
        lb = lc[c, n_a:].reshape(BLOCKS - N16, 18, 128).transpose(2, 0, 1)
        lids.append(np.concatenate(
            [la.reshape(128, N16 * 16), lb.reshape(128, -1)], axis=1
        ).astype(BF16))

    rec_all = (1.0 / np.maximum(deg, 1)).astype(np.float32)[nodemap]
    rec_all = np.ascontiguousarray(
        rec_all.reshape(N_CORES, BLOCKS, 128).transpose(0, 2, 1)
    )

    wt = np.ascontiguousarray(W.T).astype(BF16)
    bias = np.ascontiguousarray(b.reshape(128, 1))
    ident = np.eye(128, dtype=np.float32)
    iotar = np.ascontiguousarray(
        np.broadcast_to(np.arange(GW, dtype=np.float32), (128, K_MAX, GW))
        .reshape(128, K_MAX * GW)
    ).astype(BF16)

    in_maps = [
        {
            "xe": np.ascontiguousarray(xes[c]),
            "lid": np.ascontiguousarray(lids[c]),
            "recv": rec_all[c],
            "wt": wt,
            "bias": bias,
            "ident": ident,
            "iotar": iotar,
        }
        for c in range(N_CORES)
    ]
    return nodemap, in_maps


def run(edge_data, dst, W, b, trace=False, tmpdir=None):
    from concourse.bass_utils import run_bass_kernel_spmd

    nodemap, in_maps = prepare_inputs(edge_data, dst, W, b)
    nc = _get_module()
    res = run_bass_kernel_spmd(
        nc, in_maps, core_ids=list(range(N_CORES)), trace=trace, tmpdir=tmpdir,
    )
    slots = np.concatenate(
        [res.results[c]["out"].T for c in range(N_CORES)], axis=0
    ).astype(np.float32)                                   # [NPAD, 128] in slot order
    full = np.empty((NPAD, F), np.float32)
    full[nodemap] = slots
    return np.ascontiguousarray(full[:N_NODES]), res


def kernel(edge_data, dst, W, b):
    out, _ = run(edge_data, dst, W, b, trace=False)
    return out
